# revision 1
# baseline (speedup 1.0000x reference)
"""Sparse 3D convolution (gather -> matmul -> relu) for Trainium2, 8 cores.

out[n] = relu(sum_k feats[kmap[k,n]] @ W[k]), sentinel index N contributes 0.

Plan (data-parallel over voxels, no collectives):
  HOST:
    - Reorder voxels with reverse-Cuthill-McKee on the kmap adjacency so each
      voxel's 27 neighbors lie within a small band of sorted positions.
    - Each core owns NPC consecutive sorted positions; its feature slab fp64
      holds rows [base-HALO, base+NPC+HALO) padded to 64 f32/row (256B, the
      dma_gather element size), with a zero row every ZR real rows so every
      gather window starts at a zero row (sentinel target).
    - Per supertile of 1024 voxels: int16 window-local gather indices for all
      27 offsets, wrapped in dma_gather's (j%16, j//16) x8-replicated layout.
  DEVICE (per supertile):
    - dma_gather: G64[128, 216, 64] f32 <- 27648 rows (ordinal j = k*1024 +
      st*128 + p lands at partition j%128, block j//128).
    - DVE 32x32 stream-transpose of the real channels: H[32bi+c, blk*32+v] =
      G64[32bi+v, blk, c]  (one instruction, strided in-AP).
    - 27 x 4 matmuls, K=32 row-packed at tile_position (32bi, 0): W[k] is the
      stationary operand (replicated per 32-block), rhs = H[32bi:32bi+32,
      k*256:(k+1)*256]; 4 PSUM banks accumulate outT over k.
    - ACT relu PSUM -> SBUF f32, DMA out as outT[64, positions].
  HOST: un-permute rows, concatenate.
"""

import numpy as np

import concourse.bass as bass
import concourse.mybir as mybir
import concourse.tile as tile
from concourse import bacc
from concourse.bass_utils import run_bass_kernel_spmd

# --- tail-drain wait splitting -------------------------------------------
# The kernel-tail Drain carries one sem wait per engine/DMA lane still
# outstanding; walrus rejects SP CTRL instructions with multiple sync waits
# ("Too many sync wait commands"). Split the wait list across a chain of SP
# nops (one wait each) ahead of the drain.


def _split_drain_and_barrier(self, tick_clock, wait_clock):
    nc = self.nc
    collector = nc.sync.nop(nofuse=True)
    wait_clock.add_sem_waits(
        collector.ins, tile.ScopedClock({None: tick_clock.global_clock})
    )
    si = collector.ins.sync_info
    waits = list(si.on_wait) if si is not None and si.on_wait else []
    if len(waits) > 1:
        collector.ins.sync_info = mybir.SyncInfo(
            on_wait=waits[:1], on_update=list(si.on_update or [])
        )
        for w in waits[1:]:
            extra = nc.sync.nop(nofuse=True)
            extra.ins.sync_info = mybir.SyncInfo(on_wait=[w], on_update=[])
    nc.sync.drain()
    nc.all_engine_barrier()
    popped = nc._tile_sem_poison_stack.pop()
    assert popped is self._sem_poison
    nc.clear_and_free_semaphores(list(self.sems.allocated().values()))
    nc.all_engine_barrier()


tile.TileContext._drain_and_barrier = _split_drain_and_barrier

# --- problem constants ----------------------------------------------------
N = 400000
INC = 32
OUTC = 64
K3 = 27
NCORES = 8
P = 128
ES = 64  # fp64 row: 64 f32 = 256B (dma_gather element)

# device-layout constants (full problem)
NCH = 27              # gather chunks per supertile (SWDGE ring caps num_idxs at 1024)
SUPER = 1024          # voxels per supertile
NSUP = 49             # supertiles per core; 49*1024 = 50176 >= 50000
HALO = 16384
ZR = 2048             # a zero row every ZR real rows
WIN = 32768           # gather window rows
MARGIN = 15368

F32 = mybir.dt.float32
I16 = mybir.dt.int16


def _pl(u):
    """Local padded row index of local position u (zero rows at m*(ZR+1))."""
    return u + 1 + u // ZR


def _floor_zr(x):
    return (x // (ZR + 1)) * (ZR + 1)


def _bases(nsup, super_, halo, margin):
    return [max(0, _floor_zr(_pl(halo + s * super_) - margin)) for s in range(nsup)]


def build_nc(nsup, super_, fp_rows, win, bases):
    stb = super_ // P
    nidx = K3 * super_
    gblk = nidx // P  # G row-blocks per partition = K3*stb
    nc = bacc.Bacc("TRN2", target_bir_lowering=False, debug=False, num_swdge_queues=4)
    fp = nc.declare_dram_parameter("fp", [fp_rows, ES], F32, isOutput=False)
    idx = nc.declare_dram_parameter("idx", [nsup, P, nidx // 16], I16, isOutput=False)
    wrep = nc.declare_dram_parameter("wrep", [P, K3 * OUTC], F32, isOutput=False)
    outT = nc.declare_dram_parameter("outT", [OUTC, nsup * super_], F32, isOutput=True)

    with tile.TileContext(nc) as tc:
        with (
            tc.tile_pool(name="const", bufs=1) as const_pool,
            tc.tile_pool(name="idxp", bufs=2) as idx_pool,
            tc.tile_pool(name="g", bufs=2) as g_pool,
            tc.tile_pool(name="h", bufs=2) as h_pool,
            tc.tile_pool(name="o", bufs=2) as o_pool,
            tc.tile_pool(name="ps", bufs=2, space="PSUM") as psum_pool,
        ):
            w_sb = const_pool.tile([P, K3 * OUTC], F32)
            nc.sync.dma_start(out=w_sb[:], in_=wrep[:])

            for s in range(nsup):
                it = idx_pool.tile([P, nidx // 16], I16, tag="it")
                nc.sync.dma_start(out=it[:], in_=idx[s])

                G = g_pool.tile([P, gblk * ES], F32, tag="G")
                # Q7 data-scratch caps num_idxs per dma_gather (~16k int32);
                # split into NCH chunks issued across the 4 GPSIMD queues.
                cblk = gblk // NCH
                cidx = nidx // NCH
                for ci in range(NCH):
                    nc.gpsimd.dma_gather(
                        out_ap=G[:, ci * cblk * ES : (ci + 1) * cblk * ES].rearrange(
                            "p (b e) -> p b e", e=ES
                        ),
                        in_ap=fp[bases[s] : bases[s] + win],
                        idxs_ap=it[:, ci * (cidx // 16) : (ci + 1) * (cidx // 16)],
                        num_idxs=cidx,
                        num_idxs_reg=cidx,
                        elem_size=ES,
                        queue_num=ci % 4,
                    )

                H = h_pool.tile([P, gblk * INC], F32, tag="H")
                nc.vector.transpose(
                    H[:].rearrange("p (b c) -> p b c", c=INC),
                    G[:].rearrange("p (b e) -> p b e", e=ES)[:, :, 0:INC],
                )

                pbs = [
                    psum_pool.tile([OUTC, stb * 32], F32, tag=f"pb{bi}", name=f"pb{bi}")
                    for bi in range(4)
                ]
                for k in range(K3):
                    for bi in range(4):
                        nc.tensor.matmul(
                            pbs[bi][:],
                            lhsT=w_sb[32 * bi : 32 * bi + 32, k * OUTC : (k + 1) * OUTC],
                            rhs=H[
                                32 * bi : 32 * bi + 32,
                                k * stb * 32 : (k + 1) * stb * 32,
                            ],
                            start=(k == 0),
                            stop=(k == K3 - 1),
                            tile_position=(32 * bi, 0),
                        )

                o_sb = o_pool.tile([OUTC, super_], F32, tag="o")
                o_view = o_sb[:].rearrange("p (s r) -> p s r", r=P)
                for bi in range(4):
                    nc.scalar.activation(
                        out=o_view[:, :, 32 * bi : 32 * bi + 32],
                        in_=pbs[bi][:].rearrange("p (s v) -> p s v", v=32),
                        func=mybir.ActivationFunctionType.Relu,
                    )
                nc.sync.dma_start(
                    out=outT[:, s * super_ : (s + 1) * super_], in_=o_sb[:]
                )
    nc.compile()
    return nc


def rcm_order(kmap, n):
    """Bandwidth-reducing voxel order from the kmap adjacency."""
    from scipy.sparse import csr_matrix
    from scipy.sparse.csgraph import reverse_cuthill_mckee

    km = np.asarray(kmap)
    src = np.tile(np.arange(n, dtype=np.int32), K3)
    dst = km.reshape(-1).astype(np.int32)
    valid = dst < n
    src, dst = src[valid], dst[valid]
    m = csr_matrix((np.ones(src.size, dtype=np.int8), (src, dst)), shape=(n, n))
    perm = reverse_cuthill_mckee(m, symmetric_mode=True)
    return np.asarray(perm, dtype=np.int64)


def host_prep(feats, weight, kmap, ncores, nsup, super_, halo, win, bases, order):
    n = feats.shape[0]
    feats = np.asarray(feats, dtype=np.float32)
    km = np.asarray(kmap, dtype=np.int32)
    npc = nsup * super_
    nidx = K3 * super_

    rank = np.empty(n, dtype=np.int64)  # original id -> sorted position
    rank[order] = np.arange(n)
    feats_sorted = feats[order]

    # gpos[k, q]: sorted position of the k-neighbor of the voxel at sorted
    # position q (sentinel -> -1)
    km_sorted = km[:, order]
    gpos = np.where(km_sorted < n, rank[np.minimum(km_sorted, n - 1)], -1)

    band = int(np.abs(gpos - np.arange(n)[None, :])[gpos >= 0].max())
    assert band < halo - 1, f"RCM bandwidth {band} exceeds halo {halo}"

    w = np.asarray(weight, dtype=np.float32)
    wrep = (
        np.broadcast_to(w[None], (4, K3, INC, OUTC))
        .transpose(0, 2, 1, 3)
        .reshape(P, K3 * OUTC)
        .copy()
    )

    fp_rows = max(bases) + win
    base_arr = np.asarray(bases, dtype=np.int64)

    in_maps = []
    for c in range(ncores):
        lo = c * npc
        u0 = lo - halo  # global position of local position 0
        fp64 = np.zeros((fp_rows, ES), dtype=np.float32)
        gstart, gend = max(0, u0), min(n, u0 + npc + 2 * halo)
        if gend > gstart:
            us = np.arange(gstart - u0, gend - u0, dtype=np.int64)
            pls = _pl(us)
            keep = pls < fp_rows
            fp64[pls[keep], :INC] = feats_sorted[gstart:gend][keep]

        # local gather indices for this core's voxels
        q = lo + np.arange(npc)
        gp = np.where(q[None, :] < n, gpos[:, np.minimum(q, n - 1)], -1)  # [K3, npc]
        pl_idx = _pl(gp - u0)
        s_of = (np.arange(npc) // super_)[None, :]
        local = np.where(gp >= 0, pl_idx - base_arr[s_of], 0)
        assert local.min() >= 0 and local.max() < win, (
            f"window overflow: {local.min()} {local.max()}"
        )
        # ordinal j = k*super_ + r -> chunk ci = j // (nidx/NCH), then wrap
        # (jc%16, jc//16) within the chunk; chunks side by side along the
        # free dim; replicate x8 over the 128 partitions
        cidx = nidx // NCH
        js = (
            local.astype(np.int16)
            .reshape(K3, nsup, super_)
            .transpose(1, 0, 2)
            .reshape(nsup, NCH, cidx)
        )
        wrap = np.zeros((nsup, NCH, 16, cidx // 16), dtype=np.int16)
        jj = np.arange(cidx)
        wrap[:, :, jj % 16, jj // 16] = js
        wrap = wrap.transpose(0, 2, 1, 3).reshape(nsup, 16, nidx // 16)
        idx_c = np.ascontiguousarray(
            np.broadcast_to(wrap[:, None, :, :], (nsup, 8, 16, nidx // 16)).reshape(
                nsup, P, nidx // 16
            )
        )
        in_maps.append({"fp": fp64, "idx": idx_c, "wrep": wrep})
    return in_maps


def unshard(results, n, order):
    outs = [r["outT"].T for r in results]  # [npc, 64] each
    out_sorted = np.concatenate(outs, axis=0)[:n]
    out = np.empty((n, OUTC), dtype=np.float32)
    out[order] = out_sorted
    return out


def run(feats, weight, kmap, ncores, nsup, super_, halo=HALO, win=WIN,
        margin=MARGIN, **kw):
    n = feats.shape[0]
    bases = _bases(nsup, super_, halo, margin)
    fp_rows = max(bases) + win
    order = rcm_order(kmap, n)
    in_maps = host_prep(
        feats, weight, kmap, ncores, nsup, super_, halo, win, bases, order
    )
    nc = build_nc(nsup, super_, fp_rows, win, bases)
    res = run_bass_kernel_spmd(nc, in_maps, core_ids=list(range(ncores)), **kw)
    out = unshard(res.results, n, order)
    return out, res


def kernel(feats, weight, kmap):
    out, _ = run(feats, weight, kmap, NCORES, NSUP, SUPER)
    return out



# revision 2
# speedup vs baseline: 1.1320x; 1.1320x over previous
"""Sparse 3D conv (gather -> matmul -> relu) via GPSIMD ap_gather, 8 cores.

out[n] = relu(sum_k feats[kmap[k,n]] @ W[k]), sentinel index N contributes 0.

Plan (data-parallel over voxels, no collectives):
  HOST:
    - Reconstruct a raster (z-order) voxel ordering from kmap alone: BFS over
      the 26-neighbor graph propagates exact (x,y,z) offsets, so each
      connected component gets consistent coords; sort by (component, lin).
      Neighbor rank deltas are then bounded by ~3300 (measured 3292).
    - Each core owns NPC consecutive sorted voxels. Its param slab is the
      transposed feature matrix FT[32, NPC + 2*H2] (halo'd, zero-padded).
    - Per supertile of 1024 voxels: int16 window-relative gather indices for
      the 27 taps, split into 4 quarters of 7 tap-blocks (Q3 has 6 + pad).
      Invalid taps -> index 0 (a permanently-zero window column).
  DEVICE (per supertile s):
    - Rolling circular window W[128, 1+CIRC] f32 = FT columns replicated on
      4x32 partitions; col(r) = 1 + r % CIRC; fixed schedule: update s writes
      rows (s*1024+3456 .. (s+1)*1024+3456], identical ranges on all cores.
      Two alternating windows (A/B) so updates overlap gathers.
    - ONE gpsimd.ap_gather (channels=128, d=1, num_idxs=7168): each 16-lane
      Q7 core gathers its quarter's tap stream; H[128, 7*1024] lands
      matmul-ready (partition = quarter-channel, col = block*1024 + voxel).
    - 14 matmuls K=128x512 (bf16): stationary = 4 stacked tap
      weights [128, 64] (zeros for missing), rhs = H block, PSUM accumulate.
    - ACT relu PSUM -> SBUF, DMA out as outT[64, positions].
  HOST: transpose, un-permute, drop pad rows.
"""

import numpy as np

import concourse.bass as bass
import concourse.mybir as mybir
import concourse.tile as tile
from concourse import bacc, library_config
from concourse.bass_utils import run_bass_kernel_spmd

# --- tail-drain wait splitting (same workaround as baseline kernel) --------


def _split_drain_and_barrier(self, tick_clock, wait_clock):
    nc = self.nc
    collector = nc.sync.nop(nofuse=True)
    wait_clock.add_sem_waits(
        collector.ins, tile.ScopedClock({None: tick_clock.global_clock})
    )
    si = collector.ins.sync_info
    waits = list(si.on_wait) if si is not None and si.on_wait else []
    if len(waits) > 1:
        collector.ins.sync_info = mybir.SyncInfo(
            on_wait=waits[:1], on_update=list(si.on_update or [])
        )
        for w in waits[1:]:
            extra = nc.sync.nop(nofuse=True)
            extra.ins.sync_info = mybir.SyncInfo(on_wait=[w], on_update=[])
    nc.sync.drain()
    nc.all_engine_barrier()
    popped = nc._tile_sem_poison_stack.pop()
    assert popped is self._sem_poison
    nc.clear_and_free_semaphores(list(self.sems.allocated().values()))
    nc.all_engine_barrier()


tile.TileContext._drain_and_barrier = _split_drain_and_barrier

# --- problem constants ----------------------------------------------------
N = 400000
INC = 32
OUTC = 64
K3 = 27
NCORES = 8
P = 128

SUPER = 1024
NSUP = 49
NPC = NSUP * SUPER          # 50176 voxels per core
NTOT = NCORES * NPC         # 401408 padded voxel count

NBLK = 7                    # tap blocks per quarter (Q3: 6 real + 1 pad)
NIDX = NBLK * SUPER         # 7168 gather indices per Q7 core per supertile
QTAPS = [list(range(0, 7)), list(range(7, 14)),
         list(range(14, 21)), list(range(21, 27))]

BAND = 3456                 # covered reach above/below a supertile
LOW0 = -3520                # initial fill lowest row
CIRC = 8192                # circular window length (cols 1..CIRC)
WINQ = CIRC + 1             # + permanently-zero col 0
H2 = 4608                   # slab halo (>= -LOW0 and >= top margin)
SLAB = NPC + 2 * H2         # 59392 slab columns

F32 = mybir.dt.float32
F32R = mybir.dt.float32r
I16 = mybir.dt.int16


def _cover_hi(s):
    return (s + 1) * SUPER + BAND  # highest row covered after update s


def _win_slices(r0, r1):
    """Rows [r0, r1) -> list of (win_col_start, slab_rel_start, length)."""
    out = []
    r = r0
    while r < r1:
        c = 1 + (r % CIRC)
        ln = min(r1 - r, CIRC + 1 - c)
        out.append((c, r, ln))
        r += ln
    return out


def build_nc(treps=1, no_mm=False, win_elems=WINQ, bf16_mm=True):
    nc = bacc.Bacc("TRN2", target_bir_lowering=False, debug=False)
    fp = nc.declare_dram_parameter("fp", [P, SLAB], F32, isOutput=False)
    idx = nc.declare_dram_parameter("idx", [NSUP, P, NIDX // 16], I16, isOutput=False)
    wstk = nc.declare_dram_parameter("wstk", [P, NBLK * OUTC], F32, isOutput=False)
    outT = nc.declare_dram_parameter("outT", [OUTC, NPC], F32, isOutput=True)

    def upd_window(win, r0, r1):
        """DMA slab rows [r0, r1) into circular window cols (slab is already
        replicated x4 across the 128 partitions by the host)."""
        for c, r, ln in _win_slices(r0, r1):
            nc.sync.dma_start(
                out=win[:, c : c + ln], in_=fp[:, r + H2 : r + H2 + ln]
            )

    with tile.TileContext(nc) as tc:
        nc.gpsimd.load_library(library_config.ap_gather)
        with (
            tc.tile_pool(name="const", bufs=1) as const_pool,
            tc.tile_pool(name="idxp", bufs=2) as idx_pool,
            tc.tile_pool(name="h", bufs=3) as h_pool,
            tc.tile_pool(name="o", bufs=2) as o_pool,
            tc.tile_pool(name="ps", bufs=4, space="PSUM") as psum_pool,
        ):
            w_sb = const_pool.tile([P, NBLK * OUTC], F32)
            nc.sync.dma_start(out=w_sb[:], in_=wstk[:])
            if bf16_mm:
                wb16 = const_pool.tile([P, NBLK * OUTC], mybir.dt.bfloat16)
                nc.scalar.copy(out=wb16[:], in_=w_sb[:])
            else:
                wb16 = None

            wins = [const_pool.tile([P, WINQ], F32, name=f"win{i}") for i in range(2)]
            for i, w in enumerate(wins):
                nc.scalar.memzero(w[:, 0:1])

            for rep in range(treps):
                for i, w in enumerate(wins):
                    upd_window(w, LOW0, _cover_hi(i))  # win i first serves s=i
                _body(nc, tc, fp, idx, outT, w_sb, wins,
                      idx_pool, h_pool, o_pool, psum_pool, upd_window,
                      no_mm=no_mm, win_elems=win_elems, wb16=wb16)
    nc.compile()
    return nc


def _body(nc, tc, fp, idx, outT, w_sb, wins,
          idx_pool, h_pool, o_pool, psum_pool, upd_window,
          no_mm=False, win_elems=WINQ, wb16=None):
    if True:  # keep indentation shallow
        if True:
            for s in range(NSUP):
                win = wins[s % 2]
                if s >= 2:
                    # this window last served supertile s-2; roll it forward
                    upd_window(win, _cover_hi(s - 2), _cover_hi(s))

                it = idx_pool.tile([P, NIDX // 16], I16, tag="it")
                nc.scalar.dma_start(out=it[:], in_=idx[s])

                H = h_pool.tile([P, NIDX], F32, tag="H")
                nc.gpsimd.ap_gather(
                    out_ap=H[:].rearrange("p (n d) -> p n d", d=1),
                    in_ap=win[:].rearrange("p (n d) -> p n d", d=1)[:, :win_elems],
                    idxs_ap=it[:],
                    channels=P,
                    num_elems=win_elems,
                    d=1,
                    num_idxs=NIDX,
                )
                if no_mm:
                    continue
                if wb16 is not None:
                    Hb = h_pool.tile([P, NIDX], mybir.dt.bfloat16, tag="Hb")
                    nc.scalar.copy(out=Hb[:], in_=H[:])
                    Hm, Wm = Hb, wb16
                else:
                    Hm, Wm = H, w_sb

                ps = psum_pool.tile([OUTC, SUPER], F32, tag="ps")
                for h in range(2):  # matmul output must fit one PSUM bank
                    for b in range(NBLK):
                        nc.tensor.matmul(
                            ps[:, 512 * h : 512 * h + 512],
                            lhsT=Wm[:, b * OUTC : (b + 1) * OUTC],
                            rhs=Hm[:, b * SUPER + 512 * h : b * SUPER + 512 * h + 512],
                            start=(b == 0),
                            stop=(b == NBLK - 1),
                        )

                o_sb = o_pool.tile([OUTC, SUPER], F32, tag="o")
                nc.scalar.activation(
                    out=o_sb[:], in_=ps[:],
                    func=mybir.ActivationFunctionType.Relu,
                )
                nc.sync.dma_start(
                    out=outT[:, s * SUPER : (s + 1) * SUPER], in_=o_sb[:]
                )


# --- host prep ------------------------------------------------------------


def recon_order(kmap):
    """Raster voxel order reconstructed from kmap via BFS coord propagation."""
    from scipy import sparse
    from scipy.sparse import csgraph

    km = np.asarray(kmap)
    n = km.shape[1]
    offs = np.array(
        [[dx, dy, dz] for dx in (-1, 0, 1) for dy in (-1, 0, 1) for dz in (-1, 0, 1)],
        dtype=np.int32,
    )
    src = np.repeat(np.arange(n, dtype=np.int32)[None, :], K3, axis=0).ravel()
    dst = km.ravel()
    kk = np.repeat(np.arange(K3, dtype=np.int32)[:, None], n, axis=1).ravel()
    m = (dst < n) & (kk != 13)
    src, dst, kk = src[m], dst[m], kk[m]

    G = sparse.csr_matrix((np.ones(src.size, np.int8), (src, dst)), shape=(n, n))
    ncomp, labels = csgraph.connected_components(G, directed=False)

    eorder = np.argsort(src, kind="stable")
    esrc, edst, ek = src[eorder], dst[eorder], kk[eorder]
    eptr = np.searchsorted(esrc, np.arange(n + 1)).astype(np.int64)
    doff = offs[ek]

    order_scan = np.argsort(labels, kind="stable")
    starts = np.searchsorted(labels[order_scan], np.arange(ncomp))
    roots = order_scan[starts]

    coord = np.zeros((n, 3), dtype=np.int32)
    visited = np.zeros(n, dtype=bool)
    visited[roots] = True
    frontier = roots
    while frontier.size:
        cnt = eptr[frontier + 1] - eptr[frontier]
        tot = int(cnt.sum())
        if tot == 0:
            break
        base = np.repeat(eptr[frontier], cnt)
        idx = base + (np.arange(tot) - np.repeat(np.cumsum(cnt) - cnt, cnt))
        ds = edst[idx]
        ncrd = coord[np.repeat(frontier, cnt)] + doff[idx]
        fresh = ~visited[ds]
        ds_f, nc_f = ds[fresh], ncrd[fresh]
        uniq, ui = np.unique(ds_f, return_index=True)
        coord[uniq] = nc_f[ui]
        visited[uniq] = True
        frontier = uniq
    assert visited.all(), "kmap graph BFS did not reach all voxels"

    cmin = np.zeros((ncomp, 3), np.int32)
    np.minimum.at(cmin, labels, coord)
    coord -= cmin[labels]
    ext = coord.max(0).astype(np.int64) + 1
    lin_r = (coord[:, 0].astype(np.int64) * ext[1] + coord[:, 1]) * ext[2] + coord[:, 2]
    return np.lexsort((lin_r, labels))


def host_prep(feats, weight, kmap, order):
    n = feats.shape[0]
    feats = np.asarray(feats, dtype=np.float32)
    km = np.asarray(kmap, dtype=np.int32)

    rank = np.empty(n, dtype=np.int64)
    rank[order] = np.arange(n)
    feats_sorted = np.zeros((NTOT, INC), dtype=np.float32)
    feats_sorted[:n] = feats[order]

    # gpos[k, q]: sorted row of the k-tap of the voxel at sorted position q
    km_sorted = np.full((K3, NTOT), n, dtype=np.int64)
    km_sorted[:, :n] = km[:, order]
    gpos = np.where(km_sorted < n, rank[np.minimum(km_sorted, n - 1)], -1)

    band = int(np.abs(gpos - np.arange(NTOT)[None, :])[gpos >= 0].max())
    assert band < BAND, f"rank band {band} exceeds BAND {BAND}"

    # stacked weights: block b rows 32q..32q+31 = W[QTAPS[q][b]]
    w = np.asarray(weight, dtype=np.float32)
    wstk = np.zeros((P, NBLK * OUTC), dtype=np.float32)
    for q in range(4):
        for b, k in enumerate(QTAPS[q]):
            wstk[32 * q : 32 * q + 32, b * OUTC : (b + 1) * OUTC] = w[k]

    in_maps = []
    for c in range(NCORES):
        lo = c * NPC
        # slab: FT[32, SLAB], col j = sorted row lo - H2 + j
        g0, g1 = lo - H2, lo + NPC + H2
        fslab = np.zeros((SLAB, INC), dtype=np.float32)
        a, b_ = max(0, g0), min(NTOT, g1)
        fslab[a - g0 : b_ - g0] = feats_sorted[a:b_]
        fp_c = np.ascontiguousarray(np.tile(fslab.T, (4, 1)))

        # gather indices: core-local rows -> circular window cols
        gp = gpos[:, lo : lo + NPC]  # [27, NPC] absolute rows
        rloc = gp - lo
        valid = gp >= 0
        assert rloc[valid].min() >= LOW0 and rloc[valid].max() < _cover_hi(NSUP - 1)
        wcol = np.where(valid, 1 + (rloc % CIRC), 0).astype(np.int16)

        idx_c = np.zeros((NSUP, P, NIDX // 16), dtype=np.int16)
        j = np.arange(NIDX)
        for s in range(NSUP):
            for q in range(4):
                stream = np.zeros((NBLK, SUPER), dtype=np.int16)
                for b, k in enumerate(QTAPS[q]):
                    stream[b] = wcol[k, s * SUPER : (s + 1) * SUPER]
                flat = stream.reshape(-1)
                wrap = np.zeros((16, NIDX // 16), dtype=np.int16)
                wrap[j % 16, j // 16] = flat
                idx_c[s, 32 * q : 32 * q + 16] = wrap
                idx_c[s, 32 * q + 16 : 32 * q + 32] = wrap

        in_maps.append(
            {"fp": fp_c, "idx": idx_c, "wstk": wstk,
             "outT": np.zeros((OUTC, NPC), np.float32)}
        )
    return in_maps


def unshard(results, n, order):
    outs = [r["outT"].T for r in results]
    out_sorted = np.concatenate(outs, axis=0)
    out = np.empty((n, OUTC), dtype=np.float32)
    out[order] = out_sorted[:n]
    return out


_LAST_NC = None


def run(feats, weight, kmap, **kw):
    n = feats.shape[0]
    order = recon_order(kmap)
    in_maps = host_prep(feats, weight, kmap, order)
    nc = build_nc()
    res = run_bass_kernel_spmd(nc, in_maps, core_ids=list(range(NCORES)), **kw)
    out = unshard(res.results, n, order)
    return out, res


def kernel(feats, weight, kmap):
    out, _ = run(feats, weight, kmap)
    return out
